# revision 4
# baseline (speedup 1.0000x reference)
"""Axial attention block (B=8, C=512, H=W=128, 8 heads) on 8 Trainium2 cores.

Sharding: data-parallel over batch — one batch element per NeuronCore. Each
core runs both axial passes on its (C, H, W) slice and produces the full
residual sum xs + oh + ow.

Pass structure (all DMA contiguous-run; no strided DRAM access):
  - Pass 1 (HEIGHT attention, sequences along h, one per w): reads xtbf
    (C,W,H) chunks, computes oh tiles in (c, w-chunk, h) layout and writes
    them to a block-tiled DRAM scratch ohT2[hb][c, w, hi] (h = hb*8 + hi).
    The SBUF stage tile is laid out (hb, w, hi) so both DMA sides have
    >=32B/512B contiguous runs.
  - Pass 2 (WIDTH attention, sequences along w, one per h): h-chunk hb reads
    xbf + xs(f32) chunks and the matching ohT2[hb] block (contiguous), folds
    oh into the f32 residual once per chunk (GpSimd), then out = ow + resid
    per group (VectorE) and writes natural-layout output.

Matmul inputs are pre-cast to bf16 on the host (xbf natural, xtbf h<->w
swapped); weights pre-transposed to (c_in, c_out) bf16.

Per-sequence attention (S=128, dh=64): scoresT = K^T.T @ Q^T per head in
(s_k, s_q) layout, parity-split over two PSUM banks (concurrent row-group
matmuls must not share a bank); exp on ScalarE (max-subtraction skipped —
scaled scores are bounded ~7); denominators via ones-matmul on TensorE
(replicated rows); reciprocal_approx_fast + normalize on VectorE; AV packs
all 8 heads into one PSUM bank in (c, s_q) layout; O-projection batched
over 4 sequences.
"""
import numpy as np
import ml_dtypes

P = 128          # partitions
C = 512          # channels
S = 128          # sequence length (H and W)
NCB = C // P     # channel blocks
NH = 8           # heads
DH = C // NH     # head dim
G = 4            # sequences per projection group
HC1 = 16         # w-chunk, height pass
HC2 = 8          # h-chunk, width pass (= hi block size of ohT2)
HB = S // HC2    # number of h blocks
NCORES = 8

_BF16 = ml_dtypes.bfloat16

_PROG = None  # cached compiled Bass program


def _build_program():
    from contextlib import ExitStack
    import concourse.tile as tile
    from concourse import bacc, mybir

    f32 = mybir.dt.float32
    bf = mybir.dt.bfloat16
    Exp = mybir.ActivationFunctionType.Exp

    nc = bacc.Bacc("TRN2", target_bir_lowering=False, debug=False)

    xf = nc.dram_tensor("xf", [C, S, S], f32, kind="ExternalInput").ap()
    xbf = nc.dram_tensor("xbf", [C, S, S], bf, kind="ExternalInput").ap()
    xtbf = nc.dram_tensor("xtbf", [C, S, S], bf, kind="ExternalInput").ap()
    wnames = ["wq_w", "wk_w", "wv_w", "wo_w", "wq_h", "wk_h", "wv_h", "wo_h"]
    wt = {n: nc.dram_tensor(n, [C, C], bf, kind="ExternalInput").ap() for n in wnames}
    ohT2 = nc.dram_tensor("ohT2", [HB, C, S, HC2], f32).ap()
    out = nc.dram_tensor("out", [C, S, S], f32, kind="ExternalOutput").ap()

    with tile.TileContext(nc) as tc, ExitStack() as topctx:
        const = topctx.enter_context(tc.tile_pool(name="const", bufs=1))

        w_sb = {}
        for n in wnames:
            tiles = []
            for ci in range(NCB):
                t = const.tile([P, C], bf, tag=f"w_{n}_{ci}", name=f"w_{n}_{ci}")
                nc.sync.dma_start(out=t, in_=wt[n][ci * P:(ci + 1) * P, :])
                tiles.append(t)
            w_sb[n] = tiles
        ones_sb = const.tile([P, P], bf, tag="ones", name="ones")
        nc.vector.memset(ones_sb, 1.0)

        def attn_group(src_t, gsl, s0, wq, wk, wv, wo, pools):
            """One group of G sequences -> psum tiles of out-projection
            results, one (P, G*S) tile per c_out block."""
            qk_pool, vt_pool, ot_pool, et_pool, rr_pool, proj_ps, attn_ps = pools

            qt_sb, kt_sb = [], []
            for wmat, dst_list, nm in ((wq, qt_sb, "qt"), (wk, kt_sb, "kt")):
                for co in range(NCB):
                    pp = proj_ps.tile([P, G * S], f32, tag="proj", name="pp")
                    for ci in range(NCB):
                        nc.tensor.matmul(
                            pp,
                            lhsT=wmat[ci][:, co * P:(co + 1) * P],
                            rhs=src_t[ci][:, gsl, :],
                            start=(ci == 0), stop=(ci == NCB - 1))
                    sb_t = qk_pool.tile([P, G * S], bf, tag=f"{nm}{co}", name=f"{nm}{co}")
                    nc.scalar.copy(sb_t, pp)
                    dst_list.append(sb_t)

            vt_sb = []
            for sq in range(G):
                pv = proj_ps.tile([P, C], f32, tag="proj", name="pv")
                for ci in range(NCB):
                    nc.tensor.matmul(
                        pv, lhsT=src_t[ci][:, s0 + sq, :], rhs=wv[ci],
                        start=(ci == 0), stop=(ci == NCB - 1))
                vt = vt_pool.tile([P, C], bf, tag=f"vt{sq}", name=f"vt{sq}")
                nc.vector.tensor_copy(vt, pv)
                vt_sb.append(vt)

            ot_full = ot_pool.tile([P, NCB, G * S], bf, tag="ot", name="ot")
            for sq in range(G):
                ssl = slice(sq * S, (sq + 1) * S)
                stA = attn_ps.tile([P, 512], f32, tag="attn", name="stA")
                stB = attn_ps.tile([P, 512], f32, tag="attn", name="stB")
                for h in range(NH):
                    par, cb = h % 2, h // 2
                    rows = slice(par * DH, (par + 1) * DH)
                    dst = stA if par == 0 else stB
                    nc.tensor.matmul(
                        dst[:, cb * S:(cb + 1) * S],
                        lhsT=kt_sb[h // 2][rows, ssl],
                        rhs=qt_sb[h // 2][rows, ssl],
                        start=True, stop=True)
                et = et_pool.tile([P, 2, 512], bf, tag="et", name="et")
                nc.scalar.activation(out=et[:, 0, :], in_=stA, func=Exp, scale=DH ** -0.5)
                nc.scalar.activation(out=et[:, 1, :], in_=stB, func=Exp, scale=DH ** -0.5)
                rA = attn_ps.tile([P, 512], f32, tag="attn", name="rA")
                rB = attn_ps.tile([P, 512], f32, tag="attn", name="rB")
                nc.tensor.matmul(rA, lhsT=ones_sb, rhs=et[:, 0, :], start=True, stop=True)
                nc.tensor.matmul(rB, lhsT=ones_sb, rhs=et[:, 1, :], start=True, stop=True)
                rrA = rr_pool.tile([P, 512], f32, tag="rrA", name="rrA")
                rrB = rr_pool.tile([P, 512], f32, tag="rrB", name="rrB")
                nc.vector.reciprocal_approx_fast(out=rrA, in_=rA)
                nc.vector.reciprocal_approx_fast(out=rrB, in_=rB)
                etn = et_pool.tile([P, 2, 512], bf, tag="etn", name="etn")
                nc.vector.tensor_mul(etn[:, 0, :], et[:, 0, :], rrA)
                nc.vector.tensor_mul(etn[:, 1, :], et[:, 1, :], rrB)
                po = attn_ps.tile([P, 512], f32, tag="attn", name="po")
                for h in range(NH):
                    par, cb = h % 2, h // 2
                    nc.tensor.matmul(
                        po[par * DH:(par + 1) * DH, cb * S:(cb + 1) * S],
                        lhsT=vt_sb[sq][:, h * DH:(h + 1) * DH],
                        rhs=etn[:, par, cb * S:(cb + 1) * S],
                        start=True, stop=True)
                nc.scalar.copy(
                    ot_full[:, :, ssl],
                    po.rearrange("p (c s) -> p c s", c=NCB))

            pods = []
            for co in range(NCB):
                pod = proj_ps.tile([P, G * S], f32, tag="proj", name="pod")
                for ci in range(NCB):
                    nc.tensor.matmul(
                        pod,
                        lhsT=wo[ci][:, co * P:(co + 1) * P],
                        rhs=ot_full[:, ci, :],
                        start=(ci == 0), stop=(ci == NCB - 1))
                pods.append(pod)
            return pods

        def height_pass():
            """Pass 1: height attention (seq along h, one per w).  Writes oh
            to the blocked scratch ohT2[hb][c, w, hi]."""
            wq, wk, wv, wo = (w_sb["wq_h"], w_sb["wk_h"], w_sb["wv_h"], w_sb["wo_h"])
            with ExitStack() as ctx:
                src_pool = ctx.enter_context(tc.tile_pool(name="src1", bufs=2))
                stage_pool = ctx.enter_context(tc.tile_pool(name="stg1", bufs=2))
                qk_pool = ctx.enter_context(tc.tile_pool(name="qk1", bufs=2))
                vt_pool = ctx.enter_context(tc.tile_pool(name="vt1", bufs=2))
                ot_pool = ctx.enter_context(tc.tile_pool(name="ot1", bufs=2))
                et_pool = ctx.enter_context(tc.tile_pool(name="et1", bufs=2))
                rr_pool = ctx.enter_context(tc.tile_pool(name="rr1", bufs=2))
                proj_ps = ctx.enter_context(tc.tile_pool(name="pps1", bufs=2, space="PSUM"))
                attn_ps = ctx.enter_context(tc.tile_pool(name="aps1", bufs=5, space="PSUM"))
                pools = (qk_pool, vt_pool, ot_pool, et_pool, rr_pool, proj_ps, attn_ps)

                for chunk in range(S // HC1):
                    q0 = chunk * HC1
                    src_t, stage_t = [], []
                    for cb in range(NCB):
                        cs = slice(cb * P, (cb + 1) * P)
                        t = src_pool.tile([P, HC1, S], bf, tag=f"src{cb}", name=f"src{cb}")
                        nc.sync.dma_start(out=t, in_=xtbf[cs, q0:q0 + HC1, :])
                        src_t.append(t)
                        # stage layout (hb, w, hi): contiguous runs on both
                        # DMA sides of the blocked write
                        st = stage_pool.tile([P, HB, HC1, HC2], f32, tag=f"stg{cb}", name=f"stg{cb}")
                        stage_t.append(st)
                    for g in range(HC1 // G):
                        s0 = g * G
                        gsl = slice(s0, s0 + G)
                        pods = attn_group(src_t, gsl, s0, wq, wk, wv, wo, pools)
                        for co in range(NCB):
                            # pods: (p, 4 w-seq, 128 h) -> stage (hb, w in gsl, hi)
                            nc.vector.tensor_copy(
                                stage_t[co][:, :, gsl, :].rearrange("p b q i -> p q b i"),
                                pods[co].rearrange("p (q b i) -> p q b i", q=G, b=HB))
                    for cb in range(NCB):
                        cs = slice(cb * P, (cb + 1) * P)
                        nc.sync.dma_start(
                            out=ohT2[:, cs, q0:q0 + HC1, :].rearrange("b c w i -> c b w i"),
                            in_=stage_t[cb])

        def width_pass():
            """Pass 2: width attention (seq along w, one per h).  h-chunk =
            hb block; out = xs + oh + ow in natural layout."""
            wq, wk, wv, wo = (w_sb["wq_w"], w_sb["wk_w"], w_sb["wv_w"], w_sb["wo_w"])
            with ExitStack() as ctx:
                src_pool = ctx.enter_context(tc.tile_pool(name="src2", bufs=2))
                resid_pool = ctx.enter_context(tc.tile_pool(name="res2", bufs=2))
                oh_pool = ctx.enter_context(tc.tile_pool(name="oh2", bufs=2))
                stage_pool = ctx.enter_context(tc.tile_pool(name="stg2", bufs=2))
                qk_pool = ctx.enter_context(tc.tile_pool(name="qk2", bufs=2))
                vt_pool = ctx.enter_context(tc.tile_pool(name="vt2", bufs=2))
                ot_pool = ctx.enter_context(tc.tile_pool(name="ot2", bufs=2))
                et_pool = ctx.enter_context(tc.tile_pool(name="et2", bufs=2))
                rr_pool = ctx.enter_context(tc.tile_pool(name="rr2", bufs=2))
                proj_ps = ctx.enter_context(tc.tile_pool(name="pps2", bufs=2, space="PSUM"))
                attn_ps = ctx.enter_context(tc.tile_pool(name="aps2", bufs=5, space="PSUM"))
                pools = (qk_pool, vt_pool, ot_pool, et_pool, rr_pool, proj_ps, attn_ps)

                for hb in range(HB):
                    q0 = hb * HC2
                    src_t, resid_t, stage_t = [], [], []
                    for cb in range(NCB):
                        cs = slice(cb * P, (cb + 1) * P)
                        t = src_pool.tile([P, HC2, S], bf, tag=f"src{cb}", name=f"src{cb}")
                        nc.sync.dma_start(out=t, in_=xbf[cs, q0:q0 + HC2, :])
                        src_t.append(t)
                        rt = resid_pool.tile([P, HC2, S], f32, tag=f"res{cb}", name=f"res{cb}")
                        nc.sync.dma_start(out=rt, in_=xf[cs, q0:q0 + HC2, :])
                        resid_t.append(rt)
                        oht = oh_pool.tile([P, S, HC2], f32, tag=f"oh{cb}", name=f"oh{cb}")
                        nc.sync.dma_start(out=oht, in_=ohT2[hb, cs, :, :])
                        # fold oh into the residual once per chunk
                        nc.gpsimd.tensor_tensor(
                            out=rt, in0=rt,
                            in1=oht.rearrange("p w i -> p i w"),
                            op=mybir.AluOpType.add)
                        st = stage_pool.tile([P, HC2, S], f32, tag=f"stg{cb}", name=f"stg{cb}")
                        stage_t.append(st)
                    for g in range(HC2 // G):
                        s0 = g * G
                        gsl = slice(s0, s0 + G)
                        pods = attn_group(src_t, gsl, s0, wq, wk, wv, wo, pools)
                        for co in range(NCB):
                            nc.vector.tensor_add(
                                stage_t[co][:, gsl, :],
                                pods[co].rearrange("p (q s) -> p q s", q=G),
                                resid_t[co][:, gsl, :])
                    for cb in range(NCB):
                        cs = slice(cb * P, (cb + 1) * P)
                        nc.sync.dma_start(out=out[cs, q0:q0 + HC2, :], in_=stage_t[cb])

        height_pass()
        width_pass()

    nc.compile()
    return nc


def _get_program():
    global _PROG
    if _PROG is None:
        _PROG = _build_program()
    return _PROG


def kernel(xs, Wq_h, Wk_h, Wv_h, Wo_h, Wq_w, Wk_w, Wv_w, Wo_w):
    from concourse.bass_utils import run_bass_kernel_spmd

    nc = _get_program()

    wmap = {
        "wq_w": Wq_w, "wk_w": Wk_w, "wv_w": Wv_w, "wo_w": Wo_w,
        "wq_h": Wq_h, "wk_h": Wk_h, "wv_h": Wv_h, "wo_h": Wo_h,
    }
    wt_np = {n: np.ascontiguousarray(np.asarray(w, dtype=np.float32).T).astype(_BF16)
             for n, w in wmap.items()}

    xs = np.asarray(xs, dtype=np.float32)
    in_maps = []
    for b in range(NCORES):
        xb = np.ascontiguousarray(xs[b])                        # (C, H, W) f32
        xbf = xb.astype(_BF16)                                  # (C, H, W) bf16
        xtbf = np.ascontiguousarray(np.swapaxes(xb, 1, 2)).astype(_BF16)  # (C, W, H)
        in_maps.append({"xf": xb, "xbf": xbf, "xtbf": xtbf, **wt_np})

    res = run_bass_kernel_spmd(nc, in_maps, core_ids=list(range(NCORES)))
    return np.stack([res.results[b]["out"] for b in range(NCORES)], axis=0)
